# revision 28
# baseline (speedup 1.0000x reference)
"""Chamfer distance kernel for Trainium2 (8 NeuronCores, Bass/Tile).

Problem: B=4 pairs of 3-D point clouds with N=8192 points each.
  gt_pc  = coords + registration_gt   (rows  i of the distance matrix)
  gen_pc = coords + registration_pred (cols  j of the distance matrix)
  out = mean_b sum_i min_j d2[b,i,j] + mean_b sum_j min_i d2[b,i,j]

Strategy
  - Sharding: 8 cores = 4 batches x 2 row-halves (4096 rows each, all 8192
    cols). Row-mins complete per core; col-mins are min-combined across the
    2 sibling cores on the host (8192 floats per core -- negligible).
  - On-device: one augmented K=15 bf16 matmul produces squared distances
    directly in PSUM: dot(x,y) is computed as xh.yh + xl.yh + xh.yl with
    bf16 hi/lo-split coordinates (products of bf16 operands are exact in
    the PE's fp32 accumulator; the dropped xl.yl term is ~1e-5 relative),
    and both squared norms ride along as 3-way bf16 splits against ones.
    PE streams [128 x 512] tiles into PSUM (4 per 4-bank PSUM buffer).
  - Each [128 x 2048] PSUM buffer is consumed by ONE custom fused DVE op
    (see _register_chamfer_op below): colacc <- min(colacc, psum) and
    rowmin[p] <- min(seed, min_k psum[p,k]) in a single 1-elem/cycle pass.
    This is the kernel's floor: every distance element crosses the DVE
    exactly once (only the DVE can do min on PSUM data).
  - colacc [128, 8192] is partition-min-reduced with PE transposes (packed
    4 per PSUM buffer) + batched free-axis min reduces, overlapped with the
    next j-group's main work (j-group is the outer loop).
  - Host: builds the tiny [15, N] bf16 operands (O(N) work) and combines
    the per-core [128, 96] outputs into the scalar.

Measured (TRN2 instruction cost model, per core): 315.9 us total; DVE busy
~305 us vs a 273 us single-pass floor; PE ~118 us (fully overlapped).
"""

import numpy as np

import concourse.bass as bass
import concourse.mybir as mybir
from concourse import bacc
from concourse import dve_ops as _dve_ops
from concourse.dve_spec import Spec, Src0, Src1, C0, minn, lower as _dve_lower
from concourse.dve_uop import AluInp, DveOpSpec
from concourse.dve_table_gen import free_opcode_rows
from concourse.tile import TileContext
from concourse.bass_utils import run_bass_kernel_spmd

B = 4
N = 8192
N_CORES = 8
ROWS = N // 2            # rows per core
IB = ROWS // 128         # 32 i-blocks per core
GW = 2048                # DVE group width (4 PSUM banks)
JG = N // GW             # 4 j-groups
NQ = GW // 512           # 4 matmuls per group
F32 = mybir.dt.float32
BF16 = mybir.dt.bfloat16
KA = 15                  # augmented contraction dim (bf16 hi/lo split)
BIG = 3.0e38

# set by test harness to collect a profile
TRACE = False
LAST_RESULTS = None

_NC_CACHE = None


# ---------------------------------------------------------------------------
# Custom fused DVE op: one pass over a PSUM distance tile that
#   - writes  out[p,k]     = min(in0[p,k], in1[p,k])     (column-min update)
#   - reduces accum_out[p] = min(s0[p], min_k in0[p,k])  (row-min of in0 ALONE)
# The Spec language can only fold the *body* min(in0,in1) into accum_out,
# which would contaminate the row-min with column-accumulator values from
# other rows. The generated uop program carries raw Src0 on delay chain 0
# right past the accumulator block, so repointing the accumulator's stream
# input from PREV_ALU_OUT (body) to PREV_DELAY_0 (Src0) gives the
# uncontaminated fold. Lower the stock spec, apply that one-mux edit, and
# seed the compile cache so both table-gen and trace sites use it.
# ---------------------------------------------------------------------------
_OP_NAME = "CHAMFER_COLROW"


def _chamfer_ref(in0, in1, c0, c1, c2):
    P = in0.shape[0]
    x = in0.astype(np.float32)
    body = np.minimum(x, in1.astype(np.float32))
    row = x.reshape(P, -1).min(axis=-1, keepdims=True)
    return body, np.minimum(c0, row)


def _register_chamfer_op():
    for op in _dve_ops.OPS:
        if op.name == _OP_NAME:
            return op
    spec = Spec(body=minn(Src0, Src1), accum=minn, accum_init=C0,
                reference=_chamfer_ref)
    op = _dve_ops.DveOp(_OP_NAME, spec, subdim=False, uops_sha={})
    taken = set(_dve_ops._SUB_OPCODE_FOR_NAME.values())
    row = next(r for r in free_opcode_rows("TRN2") if r not in taken)
    _dve_ops.OPS.append(op)
    _dve_ops.CUSTOM_DVE_SPECS[_OP_NAME] = spec
    _dve_ops._SUB_OPCODE_FOR_NAME[_OP_NAME] = row

    uops = _dve_lower(spec, ver="v3")
    assert len(uops) == 2
    acc_blk = uops[1].datapath_config[1]
    assert acc_blk.alu_src0 == AluInp.CURR_ALU_OUT
    assert acc_blk.alu_src1 == AluInp.PREV_ALU_OUT
    acc_blk.alu_src1 = AluInp.PREV_DELAY_0  # fold raw Src0, not the body
    for u in uops:
        u.validate("v3")
    _dve_ops._COMPILE_CACHE[(_OP_NAME, "v3")] = DveOpSpec(
        name=_OP_NAME, opcode=row, uops=uops, rd1_en=True
    )
    return op


_CHAMFER_OP = _register_chamfer_op()


def _build_bass(repeat: int = 1):
    # repeat>1 duplicates the whole compute body (timing instrumentation:
    # wall(R) - wall(1) isolates HW time from dispatch overhead)
    nc = bacc.Bacc()
    a_d = nc.declare_dram_parameter("a", [KA, ROWS], BF16, isOutput=False)
    b_d = nc.declare_dram_parameter("b", [KA, N], BF16, isOutput=False)
    id_d = nc.declare_dram_parameter("ident", [128, 128], F32, isOutput=False)
    out_d = nc.declare_dram_parameter("out", [128, IB + 64], F32, isOutput=True)

    mmin = mybir.AluOpType.min

    with TileContext(nc) as tc:
        with (
            tc.tile_pool(name="const", bufs=1) as cpool,
            tc.tile_pool(name="work", bufs=1) as wpool,
            tc.tile_pool(name="ps", bufs=2, space="PSUM") as ppool,
        ):
            a_sb = cpool.tile([KA, ROWS], BF16)
            b_sb = cpool.tile([KA, N], BF16)
            ident = cpool.tile([128, 128], F32)
            # ident first: the PE's first instruction (the warm transpose
            # below) waits on it, and everything else queues behind the PE.
            # Then the slices the first matmuls need, then the rest.
            nc.sync.dma_start(out=ident[:], in_=id_d[:])
            nc.sync.dma_start(out=b_sb[:, 0:GW], in_=b_d[:, 0:GW])
            nc.sync.dma_start(out=a_sb[:, 0:ROWS // 4], in_=a_d[:, 0:ROWS // 4])
            for q in range(1, 4):
                sl = slice(q * (ROWS // 4), (q + 1) * (ROWS // 4))
                nc.sync.dma_start(out=a_sb[:, sl], in_=a_d[:, sl])
            for g in range(1, JG):
                sl = slice(g * GW, (g + 1) * GW)
                nc.sync.dma_start(out=b_sb[:, sl], in_=b_d[:, sl])

            colacc = wpool.tile([128, N], F32)
            rowmin = wpool.tile([128, IB], F32)
            colminT = wpool.tile([128, 64], F32)
            for g in range(JG):
                nc.gpsimd.memset(colacc[:, g * GW:(g + 1) * GW], BIG)

            # Make the PE observe the ident DMA queue before the main loop so
            # the real transposes at the tail don't need a 3rd sync wait
            # (walrus caps matmul wait commands at 2).
            warm = ppool.tile([128, 128], F32, tag="ps")
            nc.tensor.transpose(warm[:], ident[:], ident[:])

            # j-group outer so each group's colacc finalizes early and its
            # partition-min (transpose + reduce) overlaps the next group's
            # main work instead of forming a serial tail.
            for g in [g for _ in range(repeat) for g in range(JG)]:
                csl = colacc[:, g * GW:(g + 1) * GW]
                for ib in range(IB):
                    lhsT = a_sb[:, ib * 128:(ib + 1) * 128]
                    ps = ppool.tile([128, GW], F32, tag="ps")
                    for q in range(NQ):
                        j0 = g * GW + q * 512
                        nc.tensor.matmul(
                            ps[:, q * 512:(q + 1) * 512],
                            lhsT,
                            b_sb[:, j0:j0 + 512],
                        )
                    # fused single pass: colacc slice <- min(colacc, ps);
                    # rowmin[:, ib] <- min(seed, min_k ps) with the seed
                    # chaining the row-min across j-groups.
                    seed = BIG if g == 0 else rowmin[:, ib:ib + 1]
                    nc.vector._custom_dve(
                        _CHAMFER_OP,
                        out=csl,
                        accum_out=rowmin[:, ib:ib + 1],
                        in0=ps[:],
                        in1=csl,
                        s0=seed,
                    )
                # partition-min of this group's colacc: 16 PE transposes
                # packed 4-per-PSUM-buffer + one batched reduce per buffer
                for t4 in range(4):
                    pst = ppool.tile([128, GW], F32, tag="ps")
                    for q in range(4):
                        ck = g * 16 + t4 * 4 + q
                        nc.tensor.transpose(
                            pst[:, q * 512:q * 512 + 128],
                            colacc[:, ck * 128:(ck + 1) * 128],
                            ident[:],
                        )
                    pst3d = pst[:].rearrange("p (b r) -> p b r", b=4)[:, :, 0:128]
                    nc.vector.tensor_reduce(
                        out=colminT[:, g * 16 + t4 * 4:g * 16 + t4 * 4 + 4],
                        in_=pst3d,
                        axis=mybir.AxisListType.X,
                        op=mmin,
                    )

            nc.sync.dma_start(out=out_d[:, 0:IB], in_=rowmin[:])
            nc.sync.dma_start(out=out_d[:, IB:IB + 64], in_=colminT[:])

    nc.finalize()
    return nc


def _get_nc():
    global _NC_CACHE
    if _NC_CACHE is None:
        _NC_CACHE = _build_bass()
    return _NC_CACHE


def kernel(**inputs) -> np.ndarray:
    import ml_dtypes

    bf16 = ml_dtypes.bfloat16

    def _bf(x):
        return x.astype(bf16).astype(np.float32)

    pred = np.asarray(inputs["registration_pred"], dtype=np.float32)
    gt = np.asarray(inputs["registration_gt"], dtype=np.float32)
    coords = np.asarray(inputs["coords"], dtype=np.float32)

    gt_pc = coords + gt        # [B, 3, N]  rows (i)
    gen_pc = coords + pred     # [B, 3, N]  cols (j)
    n1 = np.sum(gt_pc * gt_pc, axis=1)    # [B, N]
    n2 = np.sum(gen_pc * gen_pc, axis=1)  # [B, N]
    ident = np.eye(128, dtype=np.float32)

    # bf16 hi/lo split: dot(x,y) ~ xh.yh + xl.yh + xh.yl (xl.yl dropped,
    # ~1e-5 relative); norms split into three bf16 terms. All products of
    # bf16 operands are exact in the PE's fp32 accumulator.
    ones = np.ones((3, N), np.float32)

    def _split3(v):  # [N] fp32 -> [3, N] bf16 triplet summing to ~v
        h = _bf(v)
        m = _bf(v - h)
        l = _bf(v - h - m)
        return np.stack([h, m, l])

    in_maps = []
    for core in range(N_CORES):
        bi, half = core // 2, core % 2
        sl = slice(half * ROWS, (half + 1) * ROWS)
        x = gt_pc[bi][:, sl]               # [3, ROWS]
        xh = _bf(x)
        xl = _bf(x - xh)
        a = np.concatenate(
            [xh, xl, xh, _split3(n1[bi][sl]), ones[:, :ROWS]], axis=0
        )                                   # [15, ROWS]
        y = gen_pc[bi]                      # [3, N]
        yh = _bf(y)
        yl = _bf(y - yh)
        bb = np.concatenate(
            [-2.0 * yh, -2.0 * yh, -2.0 * yl, ones, _split3(n2[bi])], axis=0
        )                                   # [15, N]
        in_maps.append(
            {
                "a": np.ascontiguousarray(a).astype(bf16),
                "b": np.ascontiguousarray(bb).astype(bf16),
                "ident": ident,
            }
        )

    nc = _get_nc()
    global LAST_RESULTS
    res = run_bass_kernel_spmd(
        nc, in_maps, core_ids=list(range(N_CORES)), trace=TRACE
    )
    LAST_RESULTS = res

    d1 = np.zeros(B, np.float32)
    d2 = np.zeros(B, np.float32)
    for bi in range(B):
        o0 = res.results[2 * bi]["out"]      # rows 0..4095
        o1 = res.results[2 * bi + 1]["out"]  # rows 4096..8191
        d1[bi] = o0[:, :IB].sum(dtype=np.float32) + o1[:, :IB].sum(
            dtype=np.float32
        )
        cv0 = o0[:, IB:IB + 64].T.reshape(N)
        cv1 = o1[:, IB:IB + 64].T.reshape(N)
        d2[bi] = np.minimum(cv0, cv1).sum(dtype=np.float32)

    out = np.float32(d1.mean(dtype=np.float32) + d2.mean(dtype=np.float32))
    return np.asarray(out, dtype=np.float32)


# revision 33
# speedup vs baseline: 1.0417x; 1.0417x over previous
"""Chamfer distance kernel for Trainium2 (8 NeuronCores, Bass/Tile).

Problem: B=4 pairs of 3-D point clouds with N=8192 points each.
  gt_pc  = coords + registration_gt   (rows  i of the distance matrix)
  gen_pc = coords + registration_pred (cols  j of the distance matrix)
  out = mean_b sum_i min_j d2[b,i,j] + mean_b sum_j min_i d2[b,i,j]

Strategy
  - Sharding: 8 cores = 4 batches x 2 column-halves (all 8192 rows, 4096
    cols each). Col-mins complete per core; row-min partials are
    min-combined across the 2 sibling cores on the host (8192 floats per
    core -- negligible). Column-split (not row-split) halves the on-device
    partition-min tail over the column accumulator.
  - On-device: one augmented K=15 bf16 matmul produces squared distances
    directly in PSUM: dot(x,y) is computed as xh.yh + xl.yh + xh.yl with
    bf16 hi/lo-split coordinates (products of bf16 operands are exact in
    the PE's fp32 accumulator; the dropped xl.yl term is ~1e-5 relative),
    and both squared norms ride along as 3-way bf16 splits against ones.
    PE streams [128 x 512] tiles into PSUM (4 per 4-bank PSUM buffer).
  - Each [128 x 2048] PSUM buffer is consumed by ONE custom fused DVE op
    (see _register_chamfer_op below): colacc <- min(colacc, psum) and
    rowmin[p] <- min(seed, min_k psum[p,k]) in a single 1-elem/cycle pass.
    This is the kernel's floor: every distance element crosses the DVE
    exactly once (only the DVE can do min on PSUM data).
  - colacc [128, 8192] is partition-min-reduced with PE transposes (packed
    4 per PSUM buffer) + batched free-axis min reduces, overlapped with the
    next j-group's main work (j-group is the outer loop).
  - Host: builds the tiny [15, N] bf16 operands (O(N) work) and combines
    the per-core [128, 96] outputs into the scalar.

Measured (TRN2 instruction cost model, per core): 315.9 us total; DVE busy
~305 us vs a 273 us single-pass floor; PE ~118 us (fully overlapped).
"""

import numpy as np

import concourse.bass as bass
import concourse.mybir as mybir
from concourse import bacc
from concourse import dve_ops as _dve_ops
from concourse.dve_spec import Spec, Src0, Src1, C0, minn, lower as _dve_lower
from concourse.dve_uop import AluInp, DveOpSpec
from concourse.dve_table_gen import free_opcode_rows
from concourse.tile import TileContext
from concourse.bass_utils import run_bass_kernel_spmd

B = 4
N = 8192
N_CORES = 8
COLS = N // 2            # columns per core (column-sharded: all rows local)
IB = N // 128            # 64 i-blocks per core
GW = 2048                # DVE group width (4 PSUM banks)
JG = COLS // GW          # 2 j-groups
NQ = GW // 512           # 4 matmuls per group
CKG = GW // 128          # 16 transpose chunks per group
F32 = mybir.dt.float32
BF16 = mybir.dt.bfloat16
KA = 15                  # augmented contraction dim (bf16 hi/lo split)
BIG = 3.0e38

# set by test harness to collect a profile
TRACE = False
LAST_RESULTS = None

_NC_CACHE = None


# ---------------------------------------------------------------------------
# Custom fused DVE op: one pass over a PSUM distance tile that
#   - writes  out[p,k]     = min(in0[p,k], in1[p,k])     (column-min update)
#   - reduces accum_out[p] = min(s0[p], min_k in0[p,k])  (row-min of in0 ALONE)
# The Spec language can only fold the *body* min(in0,in1) into accum_out,
# which would contaminate the row-min with column-accumulator values from
# other rows. The generated uop program carries raw Src0 on delay chain 0
# right past the accumulator block, so repointing the accumulator's stream
# input from PREV_ALU_OUT (body) to PREV_DELAY_0 (Src0) gives the
# uncontaminated fold. Lower the stock spec, apply that one-mux edit, and
# seed the compile cache so both table-gen and trace sites use it.
# ---------------------------------------------------------------------------
_OP_NAME = "CHAMFER_COLROW"


def _chamfer_ref(in0, in1, c0, c1, c2):
    P = in0.shape[0]
    x = in0.astype(np.float32)
    body = np.minimum(x, in1.astype(np.float32))
    row = x.reshape(P, -1).min(axis=-1, keepdims=True)
    return body, np.minimum(c0, row)


def _register_chamfer_op():
    for op in _dve_ops.OPS:
        if op.name == _OP_NAME:
            return op
    spec = Spec(body=minn(Src0, Src1), accum=minn, accum_init=C0,
                reference=_chamfer_ref)
    op = _dve_ops.DveOp(_OP_NAME, spec, subdim=False, uops_sha={})
    taken = set(_dve_ops._SUB_OPCODE_FOR_NAME.values())
    row = next(r for r in free_opcode_rows("TRN2") if r not in taken)
    _dve_ops.OPS.append(op)
    _dve_ops.CUSTOM_DVE_SPECS[_OP_NAME] = spec
    _dve_ops._SUB_OPCODE_FOR_NAME[_OP_NAME] = row

    uops = _dve_lower(spec, ver="v3")
    assert len(uops) == 2
    acc_blk = uops[1].datapath_config[1]
    assert acc_blk.alu_src0 == AluInp.CURR_ALU_OUT
    assert acc_blk.alu_src1 == AluInp.PREV_ALU_OUT
    acc_blk.alu_src1 = AluInp.PREV_DELAY_0  # fold raw Src0, not the body
    for u in uops:
        u.validate("v3")
    _dve_ops._COMPILE_CACHE[(_OP_NAME, "v3")] = DveOpSpec(
        name=_OP_NAME, opcode=row, uops=uops, rd1_en=True
    )
    return op


_CHAMFER_OP = _register_chamfer_op()


def _build_bass(repeat: int = 1):
    # repeat>1 duplicates the whole compute body (timing instrumentation:
    # wall(R) - wall(1) isolates HW time from dispatch overhead)
    nc = bacc.Bacc()
    a_d = nc.declare_dram_parameter("a", [KA, N], BF16, isOutput=False)
    b_d = nc.declare_dram_parameter("b", [KA, COLS], BF16, isOutput=False)
    id_d = nc.declare_dram_parameter("ident", [128, 128], F32, isOutput=False)
    out_d = nc.declare_dram_parameter(
        "out", [128, IB + COLS // 128], F32, isOutput=True
    )

    mmin = mybir.AluOpType.min

    with TileContext(nc) as tc:
        with (
            tc.tile_pool(name="const", bufs=1) as cpool,
            tc.tile_pool(name="work", bufs=1) as wpool,
            tc.tile_pool(name="ps", bufs=2, space="PSUM") as ppool,
        ):
            a_sb = cpool.tile([KA, N], BF16)
            b_sb = cpool.tile([KA, COLS], BF16)
            ident = cpool.tile([128, 128], F32)
            # ident first: the PE's first instruction (the warm transpose
            # below) waits on it, and everything else queues behind the PE.
            # Then exactly the slices the first matmuls need, then the rest.
            nc.sync.dma_start(out=ident[:], in_=id_d[:])
            nc.sync.dma_start(out=b_sb[:, 0:512], in_=b_d[:, 0:512])
            nc.sync.dma_start(out=a_sb[:, 0:512], in_=a_d[:, 0:512])
            nc.sync.dma_start(out=b_sb[:, 512:GW], in_=b_d[:, 512:GW])
            nc.sync.dma_start(out=a_sb[:, 512:GW], in_=a_d[:, 512:GW])
            for q in range(1, N // GW):
                sl = slice(q * GW, (q + 1) * GW)
                nc.sync.dma_start(out=a_sb[:, sl], in_=a_d[:, sl])
            for g in range(1, JG):
                sl = slice(g * GW, (g + 1) * GW)
                nc.sync.dma_start(out=b_sb[:, sl], in_=b_d[:, sl])

            colacc = wpool.tile([128, COLS], F32)
            rowmin = wpool.tile([128, IB], F32)
            colminT = wpool.tile([128, COLS // 128], F32)
            for g in range(JG):
                nc.gpsimd.memset(colacc[:, g * GW:(g + 1) * GW], BIG)

            # Make the PE observe the ident DMA queue before the main loop so
            # the real transposes at the tail don't need a 3rd sync wait
            # (walrus caps matmul wait commands at 2).
            warm = ppool.tile([128, 128], F32, tag="ps")
            nc.tensor.transpose(warm[:], ident[:], ident[:])

            def emit_group_tail(g):
                # partition-min of group g's colacc: CKG PE transposes packed
                # 4-per-PSUM-buffer + one batched [128, 4, 128] reduce each
                for t4 in range(CKG // 4):
                    pst = ppool.tile([128, GW], F32, tag="ps", name="pst")
                    for q in range(4):
                        ck = g * CKG + t4 * 4 + q
                        nc.tensor.transpose(
                            pst[:, q * 512:q * 512 + 128],
                            colacc[:, ck * 128:(ck + 1) * 128],
                            ident[:],
                        )
                    pst3d = pst[:].rearrange("p (b r) -> p b r", b=4)[:, :, 0:128]
                    c0 = g * CKG + t4 * 4
                    nc.vector.tensor_reduce(
                        out=colminT[:, c0:c0 + 4],
                        in_=pst3d,
                        axis=mybir.AxisListType.X,
                        op=mmin,
                    )

            # j-group outer; a finished group's tail is emitted two i-blocks
            # INTO the next group, so the PE keeps feeding the DVE new
            # distance tiles at the group boundary and does the transposes
            # in its spare cycles instead of starving the DVE.
            pending_tail = None
            groups = [g for _ in range(repeat) for g in range(JG)]
            for gi, g in enumerate(groups):
                csl = colacc[:, g * GW:(g + 1) * GW]
                for ib in range(IB):
                    lhsT = a_sb[:, ib * 128:(ib + 1) * 128]
                    ps = ppool.tile([128, GW], F32, tag="ps")
                    for q in range(NQ):
                        j0 = g * GW + q * 512
                        nc.tensor.matmul(
                            ps[:, q * 512:(q + 1) * 512],
                            lhsT,
                            b_sb[:, j0:j0 + 512],
                        )
                    # fused single pass: colacc slice <- min(colacc, ps);
                    # rowmin[:, ib] <- min(seed, min_k ps) with the seed
                    # chaining the row-min across j-groups.
                    seed = BIG if gi == 0 else rowmin[:, ib:ib + 1]
                    nc.vector._custom_dve(
                        _CHAMFER_OP,
                        out=csl,
                        accum_out=rowmin[:, ib:ib + 1],
                        in0=ps[:],
                        in1=csl,
                        s0=seed,
                    )
                    if ib == 1 and pending_tail is not None:
                        emit_group_tail(pending_tail)
                        pending_tail = None
                if gi == len(groups) - 1:
                    emit_group_tail(g)
                else:
                    pending_tail = g

            nc.sync.dma_start(out=out_d[:, 0:IB], in_=rowmin[:])
            nc.sync.dma_start(
                out=out_d[:, IB:IB + COLS // 128], in_=colminT[:]
            )

    nc.finalize()
    return nc


def _get_nc():
    global _NC_CACHE
    if _NC_CACHE is None:
        _NC_CACHE = _build_bass()
    return _NC_CACHE


def kernel(**inputs) -> np.ndarray:
    import ml_dtypes

    bf16 = ml_dtypes.bfloat16

    def _bf(x):
        return x.astype(bf16).astype(np.float32)

    pred = np.asarray(inputs["registration_pred"], dtype=np.float32)
    gt = np.asarray(inputs["registration_gt"], dtype=np.float32)
    coords = np.asarray(inputs["coords"], dtype=np.float32)

    gt_pc = coords + gt        # [B, 3, N]  rows (i)
    gen_pc = coords + pred     # [B, 3, N]  cols (j)
    n1 = np.sum(gt_pc * gt_pc, axis=1)    # [B, N]
    n2 = np.sum(gen_pc * gen_pc, axis=1)  # [B, N]
    ident = np.eye(128, dtype=np.float32)

    # bf16 hi/lo split: dot(x,y) ~ xh.yh + xl.yh + xh.yl (xl.yl dropped,
    # ~1e-5 relative); norms split into three bf16 terms. All products of
    # bf16 operands are exact in the PE's fp32 accumulator.
    ones = np.ones((3, N), np.float32)

    def _split3(v):  # [N] fp32 -> [3, N] bf16 triplet summing to ~v
        h = _bf(v)
        m = _bf(v - h)
        l = _bf(v - h - m)
        return np.stack([h, m, l])

    in_maps = []
    for core in range(N_CORES):
        bi, half = core // 2, core % 2
        sl = slice(half * COLS, (half + 1) * COLS)
        x = gt_pc[bi]                      # [3, N]  all rows on every core
        xh = _bf(x)
        xl = _bf(x - xh)
        a = np.concatenate(
            [xh, xl, xh, _split3(n1[bi]), ones], axis=0
        )                                   # [15, N]
        y = gen_pc[bi][:, sl]               # [3, COLS]  this core's columns
        yh = _bf(y)
        yl = _bf(y - yh)
        bb = np.concatenate(
            [-2.0 * yh, -2.0 * yh, -2.0 * yl, ones[:, :COLS],
             _split3(n2[bi][sl])], axis=0
        )                                   # [15, COLS]
        in_maps.append(
            {
                "a": np.ascontiguousarray(a).astype(bf16),
                "b": np.ascontiguousarray(bb).astype(bf16),
                "ident": ident,
            }
        )

    nc = _get_nc()
    global LAST_RESULTS
    res = run_bass_kernel_spmd(
        nc, in_maps, core_ids=list(range(N_CORES)), trace=TRACE
    )
    LAST_RESULTS = res

    d1 = np.zeros(B, np.float32)
    d2 = np.zeros(B, np.float32)
    nck = COLS // 128
    for bi in range(B):
        o0 = res.results[2 * bi]["out"]      # cols 0..4095
        o1 = res.results[2 * bi + 1]["out"]  # cols 4096..8191
        # row-mins are partial (per column-half): min-combine, then sum
        rm0 = o0[:, :IB].T.reshape(N)        # [p, ib] -> row ib*128+p
        rm1 = o1[:, :IB].T.reshape(N)
        d1[bi] = np.minimum(rm0, rm1).sum(dtype=np.float32)
        # col-mins are complete per core: just sum both halves
        cv0 = o0[:, IB:IB + nck].T.reshape(COLS)
        cv1 = o1[:, IB:IB + nck].T.reshape(COLS)
        d2[bi] = cv0.sum(dtype=np.float32) + cv1.sum(dtype=np.float32)

    out = np.float32(d1.mean(dtype=np.float32) + d2.mean(dtype=np.float32))
    return np.asarray(out, dtype=np.float32)


# revision 36
# speedup vs baseline: 1.0564x; 1.0141x over previous
"""Chamfer distance kernel for Trainium2 (8 NeuronCores, Bass/Tile).

Problem: B=4 pairs of 3-D point clouds with N=8192 points each.
  gt_pc  = coords + registration_gt   (rows  i of the distance matrix)
  gen_pc = coords + registration_pred (cols  j of the distance matrix)
  out = mean_b sum_i min_j d2[b,i,j] + mean_b sum_j min_i d2[b,i,j]

Strategy
  - Sharding: 8 cores = 4 batches x 2 column-halves (all 8192 rows, 4096
    cols each). Col-mins complete per core; row-min partials are
    min-combined across the 2 sibling cores on the host (8192 floats per
    core -- negligible). Column-split (not row-split) halves the on-device
    partition-min tail over the column accumulator.
  - On-device: one augmented K=15 bf16 matmul produces squared distances
    directly in PSUM: dot(x,y) is computed as xh.yh + xl.yh + xh.yl with
    bf16 hi/lo-split coordinates (products of bf16 operands are exact in
    the PE's fp32 accumulator; the dropped xl.yl term is ~1e-5 relative),
    and both squared norms ride along as 3-way bf16 splits against ones.
    PE streams [128 x 512] tiles into PSUM (4 per 4-bank PSUM buffer).
  - Each [128 x 2048] PSUM buffer is consumed by ONE custom fused DVE op
    (see _register_chamfer_op below): colacc <- min(colacc, psum) and
    rowmin[p] <- min(seed, min_k psum[p,k]) in a single 1-elem/cycle pass.
    This is the kernel's floor: every distance element crosses the DVE
    exactly once (only the DVE can do min on PSUM data).
  - colacc [128, 8192] is partition-min-reduced with PE transposes (packed
    4 per PSUM buffer) + batched free-axis min reduces, overlapped with the
    next j-group's main work (j-group is the outer loop).
  - Host: builds the tiny [15, N] bf16 operands (O(N) work) and combines
    the per-core [128, 96] outputs into the scalar.

Measured (TRN2 instruction cost model, per core): 303.2 us total; DVE busy
294 us (97% occupancy, one 1.1 us gap) vs a ~277 us single-pass floor;
PE ~118 us, fully overlapped. Every distance element crosses the DVE
exactly once -- the architectural minimum, since only the DVE can apply
min to PSUM data.
"""

import numpy as np

import concourse.bass as bass
import concourse.mybir as mybir
from concourse import bacc
from concourse import dve_ops as _dve_ops
from concourse.dve_spec import Spec, Src0, Src1, C0, minn, lower as _dve_lower
from concourse.dve_uop import AluInp, DveOpSpec
from concourse.dve_table_gen import free_opcode_rows
from concourse.tile import TileContext
from concourse.bass_utils import run_bass_kernel_spmd

B = 4
N = 8192
N_CORES = 8
COLS = N // 2            # columns per core (column-sharded: all rows local)
IB = N // 128            # 64 i-blocks per core
GW = 2048                # DVE group width (4 PSUM banks)
JG = COLS // GW          # 2 j-groups
NQ = GW // 512           # 4 matmuls per group
CKG = GW // 128          # 16 transpose chunks per group
F32 = mybir.dt.float32
BF16 = mybir.dt.bfloat16
KA = 15                  # augmented contraction dim (bf16 hi/lo split)
BIG = 3.0e38

# set by test harness to collect a profile
TRACE = False
LAST_RESULTS = None

_NC_CACHE = None


# ---------------------------------------------------------------------------
# Custom fused DVE op: one pass over a PSUM distance tile that
#   - writes  out[p,k]     = min(in0[p,k], in1[p,k])     (column-min update)
#   - reduces accum_out[p] = min(s0[p], min_k in0[p,k])  (row-min of in0 ALONE)
# The Spec language can only fold the *body* min(in0,in1) into accum_out,
# which would contaminate the row-min with column-accumulator values from
# other rows. The generated uop program carries raw Src0 on delay chain 0
# right past the accumulator block, so repointing the accumulator's stream
# input from PREV_ALU_OUT (body) to PREV_DELAY_0 (Src0) gives the
# uncontaminated fold. Lower the stock spec, apply that one-mux edit, and
# seed the compile cache so both table-gen and trace sites use it.
# ---------------------------------------------------------------------------
_OP_NAME = "CHAMFER_COLROW"


def _chamfer_ref(in0, in1, c0, c1, c2):
    P = in0.shape[0]
    x = in0.astype(np.float32)
    body = np.minimum(x, in1.astype(np.float32))
    row = x.reshape(P, -1).min(axis=-1, keepdims=True)
    return body, np.minimum(c0, row)


def _register_chamfer_op():
    for op in _dve_ops.OPS:
        if op.name == _OP_NAME:
            return op
    spec = Spec(body=minn(Src0, Src1), accum=minn, accum_init=C0,
                reference=_chamfer_ref)
    op = _dve_ops.DveOp(_OP_NAME, spec, subdim=False, uops_sha={})
    taken = set(_dve_ops._SUB_OPCODE_FOR_NAME.values())
    row = next(r for r in free_opcode_rows("TRN2") if r not in taken)
    _dve_ops.OPS.append(op)
    _dve_ops.CUSTOM_DVE_SPECS[_OP_NAME] = spec
    _dve_ops._SUB_OPCODE_FOR_NAME[_OP_NAME] = row

    uops = _dve_lower(spec, ver="v3")
    assert len(uops) == 2
    acc_blk = uops[1].datapath_config[1]
    assert acc_blk.alu_src0 == AluInp.CURR_ALU_OUT
    assert acc_blk.alu_src1 == AluInp.PREV_ALU_OUT
    acc_blk.alu_src1 = AluInp.PREV_DELAY_0  # fold raw Src0, not the body
    for u in uops:
        u.validate("v3")
    _dve_ops._COMPILE_CACHE[(_OP_NAME, "v3")] = DveOpSpec(
        name=_OP_NAME, opcode=row, uops=uops, rd1_en=True
    )
    return op


_CHAMFER_OP = _register_chamfer_op()


def _build_bass(repeat: int = 1):
    # repeat>1 duplicates the whole compute body (timing instrumentation:
    # wall(R) - wall(1) isolates HW time from dispatch overhead)
    nc = bacc.Bacc()
    a_d = nc.declare_dram_parameter("a", [KA, N], BF16, isOutput=False)
    b_d = nc.declare_dram_parameter("b", [KA, COLS], BF16, isOutput=False)
    id_d = nc.declare_dram_parameter("ident", [128, 128], F32, isOutput=False)
    out_d = nc.declare_dram_parameter(
        "out", [128, IB + COLS // 128], F32, isOutput=True
    )

    mmin = mybir.AluOpType.min

    with TileContext(nc) as tc:
        with (
            tc.tile_pool(name="const", bufs=1) as cpool,
            tc.tile_pool(name="work", bufs=1) as wpool,
            tc.tile_pool(name="stage", bufs=3) as spool,
            tc.tile_pool(name="ps", bufs=2, space="PSUM") as ppool,
        ):
            a_sb = cpool.tile([KA, N], BF16)
            b_sb = cpool.tile([KA, COLS], BF16)
            ident = cpool.tile([128, 128], F32)
            # ident first: the PE's first instruction (the warm transpose
            # below) waits on it, and everything else queues behind the PE.
            # Then exactly the slices the first matmuls need, then the rest.
            nc.sync.dma_start(out=ident[:], in_=id_d[:])
            nc.sync.dma_start(out=b_sb[:, 0:512], in_=b_d[:, 0:512])
            nc.sync.dma_start(out=a_sb[:, 0:512], in_=a_d[:, 0:512])
            nc.sync.dma_start(out=b_sb[:, 512:GW], in_=b_d[:, 512:GW])
            nc.sync.dma_start(out=a_sb[:, 512:GW], in_=a_d[:, 512:GW])
            for q in range(1, N // GW):
                sl = slice(q * GW, (q + 1) * GW)
                nc.sync.dma_start(out=a_sb[:, sl], in_=a_d[:, sl])
            for g in range(1, JG):
                sl = slice(g * GW, (g + 1) * GW)
                nc.sync.dma_start(out=b_sb[:, sl], in_=b_d[:, sl])

            colacc = wpool.tile([128, COLS], F32)
            rowmin = wpool.tile([128, IB], F32)
            colminT = wpool.tile([128, COLS // 128], F32)
            for g in range(JG):
                nc.gpsimd.memset(colacc[:, g * GW:(g + 1) * GW], BIG)

            # Make the PE observe the ident DMA queue before the main loop so
            # the real transposes at the tail don't need a 3rd sync wait
            # (walrus caps matmul wait commands at 2).
            warm = ppool.tile([128, 128], F32, tag="ps")
            nc.tensor.transpose(warm[:], ident[:], ident[:])

            def emit_group_tail(g):
                # partition-min of group g's colacc: CKG PE transposes packed
                # 4-per-PSUM-buffer + one batched [128, 4, 128] reduce each
                for t4 in range(CKG // 4):
                    pst = ppool.tile([128, GW], F32, tag="ps", name="pst")
                    for q in range(4):
                        ck = g * CKG + t4 * 4 + q
                        nc.tensor.transpose(
                            pst[:, q * 512:q * 512 + 128],
                            colacc[:, ck * 128:(ck + 1) * 128],
                            ident[:],
                        )
                    pst3d = pst[:].rearrange("p (b r) -> p b r", b=4)[:, :, 0:128]
                    c0 = g * CKG + t4 * 4
                    nc.vector.tensor_reduce(
                        out=colminT[:, c0:c0 + 4],
                        in_=pst3d,
                        axis=mybir.AxisListType.X,
                        op=mmin,
                    )

            # j-group outer; a finished group's tail is emitted two i-blocks
            # INTO the next group, so the PE keeps feeding the DVE new
            # distance tiles at the group boundary and does the transposes
            # in its spare cycles instead of starving the DVE.
            pending_tail = None
            groups = [g for _ in range(repeat) for g in range(JG)]
            for gi, g in enumerate(groups):
                csl = colacc[:, g * GW:(g + 1) * GW]
                for ib in range(IB):
                    lhsT = a_sb[:, ib * 128:(ib + 1) * 128]
                    ps = ppool.tile([128, GW], F32, tag="ps")
                    for q in range(NQ):
                        j0 = g * GW + q * 512
                        nc.tensor.matmul(
                            ps[:, q * 512:(q + 1) * 512],
                            lhsT,
                            b_sb[:, j0:j0 + 512],
                        )
                    # Stage the tile PSUM->SBUF on the otherwise-idle ScalarE:
                    # the DVE's per-op constant drops from 120 (PSUM src) to
                    # 58 cycles (SBUF src), ~8us over the kernel. ACT runs at
                    # 1.85us/tile vs the DVE's 2.2us -- it keeps ahead.
                    stage = spool.tile([128, GW], F32, tag="stage")
                    nc.scalar.copy(stage[:], ps[:])
                    # fused single pass: colacc slice <- min(colacc, stage);
                    # rowmin[:, ib] <- min(seed, min_k stage) with the seed
                    # chaining the row-min across j-groups.
                    seed = BIG if gi == 0 else rowmin[:, ib:ib + 1]
                    nc.vector._custom_dve(
                        _CHAMFER_OP,
                        out=csl,
                        accum_out=rowmin[:, ib:ib + 1],
                        in0=stage[:],
                        in1=csl,
                        s0=seed,
                    )
                    if ib == 1 and pending_tail is not None:
                        emit_group_tail(pending_tail)
                        pending_tail = None
                if gi == len(groups) - 1:
                    emit_group_tail(g)
                else:
                    pending_tail = g

            nc.sync.dma_start(out=out_d[:, 0:IB], in_=rowmin[:])
            nc.sync.dma_start(
                out=out_d[:, IB:IB + COLS // 128], in_=colminT[:]
            )

    nc.finalize()
    return nc


def _get_nc():
    global _NC_CACHE
    if _NC_CACHE is None:
        _NC_CACHE = _build_bass()
    return _NC_CACHE


def kernel(**inputs) -> np.ndarray:
    import ml_dtypes

    bf16 = ml_dtypes.bfloat16

    def _bf(x):
        return x.astype(bf16).astype(np.float32)

    pred = np.asarray(inputs["registration_pred"], dtype=np.float32)
    gt = np.asarray(inputs["registration_gt"], dtype=np.float32)
    coords = np.asarray(inputs["coords"], dtype=np.float32)

    gt_pc = coords + gt        # [B, 3, N]  rows (i)
    gen_pc = coords + pred     # [B, 3, N]  cols (j)
    n1 = np.sum(gt_pc * gt_pc, axis=1)    # [B, N]
    n2 = np.sum(gen_pc * gen_pc, axis=1)  # [B, N]
    ident = np.eye(128, dtype=np.float32)

    # bf16 hi/lo split: dot(x,y) ~ xh.yh + xl.yh + xh.yl (xl.yl dropped,
    # ~1e-5 relative); norms split into three bf16 terms. All products of
    # bf16 operands are exact in the PE's fp32 accumulator.
    ones = np.ones((3, N), np.float32)

    def _split3(v):  # [N] fp32 -> [3, N] bf16 triplet summing to ~v
        h = _bf(v)
        m = _bf(v - h)
        l = _bf(v - h - m)
        return np.stack([h, m, l])

    in_maps = []
    for core in range(N_CORES):
        bi, half = core // 2, core % 2
        sl = slice(half * COLS, (half + 1) * COLS)
        x = gt_pc[bi]                      # [3, N]  all rows on every core
        xh = _bf(x)
        xl = _bf(x - xh)
        a = np.concatenate(
            [xh, xl, xh, _split3(n1[bi]), ones], axis=0
        )                                   # [15, N]
        y = gen_pc[bi][:, sl]               # [3, COLS]  this core's columns
        yh = _bf(y)
        yl = _bf(y - yh)
        bb = np.concatenate(
            [-2.0 * yh, -2.0 * yh, -2.0 * yl, ones[:, :COLS],
             _split3(n2[bi][sl])], axis=0
        )                                   # [15, COLS]
        in_maps.append(
            {
                "a": np.ascontiguousarray(a).astype(bf16),
                "b": np.ascontiguousarray(bb).astype(bf16),
                "ident": ident,
            }
        )

    nc = _get_nc()
    global LAST_RESULTS
    res = run_bass_kernel_spmd(
        nc, in_maps, core_ids=list(range(N_CORES)), trace=TRACE
    )
    LAST_RESULTS = res

    d1 = np.zeros(B, np.float32)
    d2 = np.zeros(B, np.float32)
    nck = COLS // 128
    for bi in range(B):
        o0 = res.results[2 * bi]["out"]      # cols 0..4095
        o1 = res.results[2 * bi + 1]["out"]  # cols 4096..8191
        # row-mins are partial (per column-half): min-combine, then sum
        rm0 = o0[:, :IB].T.reshape(N)        # [p, ib] -> row ib*128+p
        rm1 = o1[:, :IB].T.reshape(N)
        d1[bi] = np.minimum(rm0, rm1).sum(dtype=np.float32)
        # col-mins are complete per core: just sum both halves
        cv0 = o0[:, IB:IB + nck].T.reshape(COLS)
        cv1 = o1[:, IB:IB + nck].T.reshape(COLS)
        d2[bi] = cv0.sum(dtype=np.float32) + cv1.sum(dtype=np.float32)

    out = np.float32(d1.mean(dtype=np.float32) + d2.mean(dtype=np.float32))
    return np.asarray(out, dtype=np.float32)


# revision 39
# speedup vs baseline: 1.0613x; 1.0046x over previous
"""Chamfer distance kernel for Trainium2 (8 NeuronCores, Bass/Tile).

Problem: B=4 pairs of 3-D point clouds with N=8192 points each.
  gt_pc  = coords + registration_gt   (rows  i of the distance matrix)
  gen_pc = coords + registration_pred (cols  j of the distance matrix)
  out = mean_b sum_i min_j d2[b,i,j] + mean_b sum_j min_i d2[b,i,j]

Strategy
  - Sharding: 8 cores = 4 batches x 2 column-halves (all 8192 rows, 4096
    cols each). Col-mins complete per core; row-min partials are
    min-combined across the 2 sibling cores on the host (8192 floats per
    core -- negligible). Column-split (not row-split) halves the on-device
    partition-min tail over the column accumulator.
  - On-device: one augmented K=15 bf16 matmul produces squared distances
    directly in PSUM: dot(x,y) is computed as xh.yh + xl.yh + xh.yl with
    bf16 hi/lo-split coordinates (products of bf16 operands are exact in
    the PE's fp32 accumulator; the dropped xl.yl term is ~1e-5 relative),
    and both squared norms ride along as 3-way bf16 splits against ones.
    PE streams [128 x 512] tiles into PSUM (4 per 4-bank PSUM buffer).
  - Each [128 x 2048] PSUM buffer is consumed by ONE custom fused DVE op
    (see _register_chamfer_op below): colacc <- min(colacc, psum) and
    rowmin[p] <- min(seed, min_k psum[p,k]) in a single 1-elem/cycle pass.
    This is the kernel's floor: every distance element crosses the DVE
    exactly once (only the DVE can do min on PSUM data).
  - colacc [128, 8192] is partition-min-reduced with PE transposes (packed
    4 per PSUM buffer) + batched free-axis min reduces, overlapped with the
    next j-group's main work (j-group is the outer loop).
  - Host: builds the tiny [15, N] bf16 operands (O(N) work) and combines
    the per-core [128, 96] outputs into the scalar.

Measured (TRN2 instruction cost model, per core): 303.2 us total; DVE busy
294 us (97% occupancy, one 1.1 us gap) vs a ~277 us single-pass floor;
PE ~118 us, fully overlapped. Every distance element crosses the DVE
exactly once -- the architectural minimum, since only the DVE can apply
min to PSUM data.
"""

import numpy as np

import concourse.bass as bass
import concourse.mybir as mybir
from concourse import bacc
from concourse import dve_ops as _dve_ops
from concourse.dve_spec import Spec, Src0, Src1, C0, minn, lower as _dve_lower
from concourse.dve_uop import AluInp, DveOpSpec
from concourse.dve_table_gen import free_opcode_rows
from concourse.tile import TileContext
from concourse.bass_utils import run_bass_kernel_spmd

B = 4
N = 8192
N_CORES = 8
COLS = N // 2            # columns per core (column-sharded: all rows local)
IB = N // 128            # 64 i-blocks per core
GW = 2048                # DVE group width (4 PSUM banks)
JG = COLS // GW          # 2 j-groups
NQ = GW // 512           # 4 matmuls per group
CKG = GW // 128          # 16 transpose chunks per group
F32 = mybir.dt.float32
BF16 = mybir.dt.bfloat16
KA = 15                  # augmented contraction dim (bf16 hi/lo split)
BIG = 3.0e38

# set by test harness to collect a profile
TRACE = False
LAST_RESULTS = None

_NC_CACHE = None


# ---------------------------------------------------------------------------
# Custom fused DVE op: one pass over a PSUM distance tile that
#   - writes  out[p,k]     = min(in0[p,k], in1[p,k])     (column-min update)
#   - reduces accum_out[p] = min(s0[p], min_k in0[p,k])  (row-min of in0 ALONE)
# The Spec language can only fold the *body* min(in0,in1) into accum_out,
# which would contaminate the row-min with column-accumulator values from
# other rows. The generated uop program carries raw Src0 on delay chain 0
# right past the accumulator block, so repointing the accumulator's stream
# input from PREV_ALU_OUT (body) to PREV_DELAY_0 (Src0) gives the
# uncontaminated fold. Lower the stock spec, apply that one-mux edit, and
# seed the compile cache so both table-gen and trace sites use it.
# ---------------------------------------------------------------------------
_OP_NAME = "CHAMFER_COLROW"


def _chamfer_ref(in0, in1, c0, c1, c2):
    P = in0.shape[0]
    x = in0.astype(np.float32)
    body = np.minimum(x, in1.astype(np.float32))
    row = x.reshape(P, -1).min(axis=-1, keepdims=True)
    return body, np.minimum(c0, row)


def _register_chamfer_op():
    for op in _dve_ops.OPS:
        if op.name == _OP_NAME:
            return op
    spec = Spec(body=minn(Src0, Src1), accum=minn, accum_init=C0,
                reference=_chamfer_ref)
    op = _dve_ops.DveOp(_OP_NAME, spec, subdim=False, uops_sha={})
    taken = set(_dve_ops._SUB_OPCODE_FOR_NAME.values())
    row = next(r for r in free_opcode_rows("TRN2") if r not in taken)
    _dve_ops.OPS.append(op)
    _dve_ops.CUSTOM_DVE_SPECS[_OP_NAME] = spec
    _dve_ops._SUB_OPCODE_FOR_NAME[_OP_NAME] = row

    uops = _dve_lower(spec, ver="v3")
    assert len(uops) == 2
    acc_blk = uops[1].datapath_config[1]
    assert acc_blk.alu_src0 == AluInp.CURR_ALU_OUT
    assert acc_blk.alu_src1 == AluInp.PREV_ALU_OUT
    acc_blk.alu_src1 = AluInp.PREV_DELAY_0  # fold raw Src0, not the body
    for u in uops:
        u.validate("v3")
    _dve_ops._COMPILE_CACHE[(_OP_NAME, "v3")] = DveOpSpec(
        name=_OP_NAME, opcode=row, uops=uops, rd1_en=True
    )
    return op


_CHAMFER_OP = _register_chamfer_op()


def _build_bass(repeat: int = 1):
    # repeat>1 duplicates the whole compute body (timing instrumentation:
    # wall(R) - wall(1) isolates HW time from dispatch overhead)
    nc = bacc.Bacc()
    a_d = nc.declare_dram_parameter("a", [KA, N], BF16, isOutput=False)
    b_d = nc.declare_dram_parameter("b", [KA, COLS], BF16, isOutput=False)
    id_d = nc.declare_dram_parameter("ident", [128, 128], F32, isOutput=False)
    out_d = nc.declare_dram_parameter(
        "out", [128, IB + COLS // 128], F32, isOutput=True
    )

    mmin = mybir.AluOpType.min

    with TileContext(nc) as tc:
        with (
            tc.tile_pool(name="const", bufs=1) as cpool,
            tc.tile_pool(name="work", bufs=1) as wpool,
            tc.tile_pool(name="stage", bufs=3) as spool,
            tc.tile_pool(name="ps", bufs=2, space="PSUM") as ppool,
        ):
            a_sb = cpool.tile([KA, N], BF16)
            b_sb = cpool.tile([KA, COLS], BF16)
            ident = cpool.tile([128, 128], F32)
            # ident first: the PE's first instruction (the warm transpose
            # below) waits on it, and everything else queues behind the PE.
            # Then exactly the slices the first matmuls need, then the rest.
            nc.sync.dma_start(out=ident[:], in_=id_d[:])
            nc.sync.dma_start(out=b_sb[:, 0:512], in_=b_d[:, 0:512])
            nc.sync.dma_start(out=a_sb[:, 0:512], in_=a_d[:, 0:512])
            nc.sync.dma_start(out=b_sb[:, 512:GW], in_=b_d[:, 512:GW])
            nc.sync.dma_start(out=a_sb[:, 512:GW], in_=a_d[:, 512:GW])
            for q in range(1, N // GW):
                sl = slice(q * GW, (q + 1) * GW)
                nc.sync.dma_start(out=a_sb[:, sl], in_=a_d[:, sl])
            for g in range(1, JG):
                sl = slice(g * GW, (g + 1) * GW)
                nc.sync.dma_start(out=b_sb[:, sl], in_=b_d[:, sl])

            colacc = wpool.tile([128, COLS], F32)
            rowmin = wpool.tile([128, IB], F32)
            colminT = wpool.tile([128, COLS // 128], F32)
            for g in range(JG):
                nc.gpsimd.memset(colacc[:, g * GW:(g + 1) * GW], BIG)

            # Make the PE observe the ident DMA queue before the main loop so
            # the real transposes at the tail don't need a 3rd sync wait
            # (walrus caps matmul wait commands at 2).
            warm = ppool.tile([128, 128], F32, tag="ps")
            nc.tensor.transpose(warm[:], ident[:], ident[:])

            def emit_tail_batch(g, t4):
                # partition-min of 4 chunks of group g's colacc: 4 packed PE
                # transposes + one batched [128, 4, 128] min reduce
                pst = ppool.tile([128, GW], F32, tag="ps", name="pst")
                for q in range(4):
                    ck = g * CKG + t4 * 4 + q
                    nc.tensor.transpose(
                        pst[:, q * 512:q * 512 + 128],
                        colacc[:, ck * 128:(ck + 1) * 128],
                        ident[:],
                    )
                pst3d = pst[:].rearrange("p (b r) -> p b r", b=4)[:, :, 0:128]
                c0 = g * CKG + t4 * 4
                nc.vector.tensor_reduce(
                    out=colminT[:, c0:c0 + 4],
                    in_=pst3d,
                    axis=mybir.AxisListType.X,
                    op=mmin,
                )

            # j-group outer; a finished group's tail is emitted two i-blocks
            # INTO the next group, so the PE keeps feeding the DVE new
            # distance tiles at the group boundary and does the transposes
            # in its spare cycles instead of starving the DVE.
            pending_tail = None
            groups = [g for _ in range(repeat) for g in range(JG)]
            for gi, g in enumerate(groups):
                csl = colacc[:, g * GW:(g + 1) * GW]
                for ib in range(IB):
                    lhsT = a_sb[:, ib * 128:(ib + 1) * 128]
                    ps = ppool.tile([128, GW], F32, tag="ps")
                    for q in range(NQ):
                        j0 = g * GW + q * 512
                        nc.tensor.matmul(
                            ps[:, q * 512:(q + 1) * 512],
                            lhsT,
                            b_sb[:, j0:j0 + 512],
                        )
                    # Stage the tile PSUM->SBUF on the otherwise-idle ScalarE:
                    # the DVE's per-op constant drops from 120 (PSUM src) to
                    # 58 cycles (SBUF src), ~8us over the kernel. ACT runs at
                    # 1.85us/tile vs the DVE's 2.2us -- it keeps ahead. The
                    # very first tile skips staging so the DVE starts ~1.8us
                    # sooner (one 120-cycle PSUM-src op costs nothing there).
                    if gi == 0 and ib == 0:
                        stage = ps
                    else:
                        stage = spool.tile([128, GW], F32, tag="stage")
                        nc.scalar.copy(stage[:], ps[:])
                    # fused single pass: colacc slice <- min(colacc, stage);
                    # rowmin[:, ib] <- min(seed, min_k stage) with the seed
                    # chaining the row-min across j-groups.
                    seed = BIG if gi == 0 else rowmin[:, ib:ib + 1]
                    nc.vector._custom_dve(
                        _CHAMFER_OP,
                        out=csl,
                        accum_out=rowmin[:, ib:ib + 1],
                        in0=stage[:],
                        in1=csl,
                        s0=seed,
                    )
                    # a finished group's 4 tail batches are spread across the
                    # next group (one per few i-blocks) so each batch's brief
                    # PSUM-slot steal hides in the MM pipeline's slack
                    if pending_tail is not None and ib in (1, 5, 9, 13):
                        emit_tail_batch(pending_tail, (ib - 1) // 4)
                        if ib == 13:
                            pending_tail = None
                if gi == len(groups) - 1:
                    for t4 in range(CKG // 4):
                        emit_tail_batch(g, t4)
                else:
                    pending_tail = g

            nc.sync.dma_start(out=out_d[:, 0:IB], in_=rowmin[:])
            nc.sync.dma_start(
                out=out_d[:, IB:IB + COLS // 128], in_=colminT[:]
            )

    nc.finalize()
    return nc


def _get_nc():
    global _NC_CACHE
    if _NC_CACHE is None:
        _NC_CACHE = _build_bass()
    return _NC_CACHE


def kernel(**inputs) -> np.ndarray:
    import ml_dtypes

    bf16 = ml_dtypes.bfloat16

    def _bf(x):
        return x.astype(bf16).astype(np.float32)

    pred = np.asarray(inputs["registration_pred"], dtype=np.float32)
    gt = np.asarray(inputs["registration_gt"], dtype=np.float32)
    coords = np.asarray(inputs["coords"], dtype=np.float32)

    gt_pc = coords + gt        # [B, 3, N]  rows (i)
    gen_pc = coords + pred     # [B, 3, N]  cols (j)
    n1 = np.sum(gt_pc * gt_pc, axis=1)    # [B, N]
    n2 = np.sum(gen_pc * gen_pc, axis=1)  # [B, N]
    ident = np.eye(128, dtype=np.float32)

    # bf16 hi/lo split: dot(x,y) ~ xh.yh + xl.yh + xh.yl (xl.yl dropped,
    # ~1e-5 relative); norms split into three bf16 terms. All products of
    # bf16 operands are exact in the PE's fp32 accumulator.
    ones = np.ones((3, N), np.float32)

    def _split3(v):  # [N] fp32 -> [3, N] bf16 triplet summing to ~v
        h = _bf(v)
        m = _bf(v - h)
        l = _bf(v - h - m)
        return np.stack([h, m, l])

    in_maps = []
    for core in range(N_CORES):
        bi, half = core // 2, core % 2
        sl = slice(half * COLS, (half + 1) * COLS)
        x = gt_pc[bi]                      # [3, N]  all rows on every core
        xh = _bf(x)
        xl = _bf(x - xh)
        a = np.concatenate(
            [xh, xl, xh, _split3(n1[bi]), ones], axis=0
        )                                   # [15, N]
        y = gen_pc[bi][:, sl]               # [3, COLS]  this core's columns
        yh = _bf(y)
        yl = _bf(y - yh)
        bb = np.concatenate(
            [-2.0 * yh, -2.0 * yh, -2.0 * yl, ones[:, :COLS],
             _split3(n2[bi][sl])], axis=0
        )                                   # [15, COLS]
        in_maps.append(
            {
                "a": np.ascontiguousarray(a).astype(bf16),
                "b": np.ascontiguousarray(bb).astype(bf16),
                "ident": ident,
            }
        )

    nc = _get_nc()
    global LAST_RESULTS
    res = run_bass_kernel_spmd(
        nc, in_maps, core_ids=list(range(N_CORES)), trace=TRACE
    )
    LAST_RESULTS = res

    d1 = np.zeros(B, np.float32)
    d2 = np.zeros(B, np.float32)
    nck = COLS // 128
    for bi in range(B):
        o0 = res.results[2 * bi]["out"]      # cols 0..4095
        o1 = res.results[2 * bi + 1]["out"]  # cols 4096..8191
        # row-mins are partial (per column-half): min-combine, then sum
        rm0 = o0[:, :IB].T.reshape(N)        # [p, ib] -> row ib*128+p
        rm1 = o1[:, :IB].T.reshape(N)
        d1[bi] = np.minimum(rm0, rm1).sum(dtype=np.float32)
        # col-mins are complete per core: just sum both halves
        cv0 = o0[:, IB:IB + nck].T.reshape(COLS)
        cv1 = o1[:, IB:IB + nck].T.reshape(COLS)
        d2[bi] = cv0.sum(dtype=np.float32) + cv1.sum(dtype=np.float32)

    out = np.float32(d1.mean(dtype=np.float32) + d2.mean(dtype=np.float32))
    return np.asarray(out, dtype=np.float32)
